# revision 23
# baseline (speedup 1.0000x reference)
"""Masked-softmax cross-entropy loss on 8 Trainium2 cores.

Math: for each target row t (16384 rows of length 4096):
  numer[t] = sum_j exp(x[t,j]/tau) over valid src cols j whose color == tgt color t
  denom[t] = sum_j exp(x[t,j]/tau) over valid src cols j
  p_gt = numer/denom, nll = -log(p_gt + eps), rows with numer==0 are masked out.
Segment/count aggregation (32 segments) happens on host - it touches 16K scalars.

Sharding: core c takes half a batch: batch c//2, row-half c%2 (2048 rows).
All rows on a core share one batch => one src color-id row + one valid mask.

Device pipeline per 128-row tile (i = 0..15):
  DMA  (swdge): load x tile [128,4096] f32 (contiguous 2MB)
  Pool/DVE:     x += vbias   (vbias[j] = -1e30 on invalid src cols, else 0)
  ScalarE:      et = exp(10*x), accum_out -> denom (invalid cols exp to 0)
  DVE:          scalar_tensor_tensor fused compare-multiply-accumulate:
                   numer = sum((src_id == tgt_id[t]) * et)  per partition
Colors are mapped to small integer ids on host (exact byte equality), so an
f32 equality compare on device reproduces the reference's exact color match.
src pad -> id -1, tgt pad -> id -2 (never matches; -1 cols get vbias -1e30).

Sync-wait budget: this walrus allows few sem waits per instruction (1 for
STT/CTRL). Tiny same-engine "interposer" copies absorb cross-engine waits,
and the kernel-tail drain is split into one drain per proc.
"""

import os
import numpy as np

B = 4
S_TGT = 8
L_TGT = 512
C = 4
N = 4096          # src columns (= 8*512), also total tgt rows per batch
P = 128
ROWS = 2048       # tgt rows per core (half a batch)
NTILES = ROWS // P  # 16
NBUF = 3          # x/exps tile buffer depth (slot reuse distance)
NCORES = 8
PAD = -1.0
EPS = 1e-15
MASKBIAS = -1e30

# tiles whose mask-add runs on the Pool engine (rest on vector); tuned from
# profile to balance Pool TT rate (0.42 eff => ~8.1us/tile) vs DVE
MASK_POOL_TILES = frozenset((0, 2, 4, 6, 8, 9, 11, 13, 15))

# engine issuing the big x-tile loads ("sync" = HWDGE, "gpsimd" = SWDGE);
# HWDGE DMACopy allows only one sync wait under this walrus, SWDGE is fine
LOAD_ENGINE = "gpsimd"

_NC_CACHE = {}


def _patch_split_drain():
    """Split the kernel-tail drain's sem waits across several drain
    instructions (walrus rejects >1 sync wait on one CTRL instruction)."""
    import concourse.tile as tile
    from concourse.vector_clock import ScopedClock, VectorClock

    if getattr(tile.TileContext, "_split_drain_patched", False):
        return

    def _drain_and_barrier(self, tick_clock, wait_clock):
        g = tick_clock.global_clock
        n = len(g)
        for base in range(n):
            vec = [g[i] if i == base else 0 for i in range(n)]
            if not any(vec):
                continue
            d = self.nc.sync.drain()
            wait_clock.add_sem_waits(d.ins, ScopedClock({None: VectorClock(vec)}))
        self.nc.all_engine_barrier()
        popped = self.nc._tile_sem_poison_stack.pop()
        assert popped is self._sem_poison
        self.nc.clear_and_free_semaphores(list(self.sems.allocated().values()))
        self.nc.all_engine_barrier()

    tile.TileContext._drain_and_barrier = _drain_and_barrier
    tile.TileContext._split_drain_patched = True


def _build_nc():
    import concourse.bass as bass
    import concourse.mybir as mybir
    import concourse.tile as tile
    from concourse.tile_rust import add_dep_helper
    from contextlib import ExitStack

    _patch_split_drain()
    nc = bass.Bass()
    dt = mybir.dt.float32
    x = nc.declare_dram_parameter("x", [ROWS, N], dt, isOutput=False)
    src_ids = nc.declare_dram_parameter("src_ids", [P, N], dt, isOutput=False)
    vbias = nc.declare_dram_parameter("vbias", [P, N], dt, isOutput=False)
    tgt_ids = nc.declare_dram_parameter("tgt_ids", [P, NTILES], dt, isOutput=False)
    numer = nc.declare_dram_parameter("numer", [P, NTILES], dt, isOutput=True)
    denom = nc.declare_dram_parameter("denom", [P, NTILES], dt, isOutput=True)

    with tile.TileContext(nc) as tc:
        with ExitStack() as ctx:
            const_pool = ctx.enter_context(tc.tile_pool(name="const", bufs=1))
            x_pool = ctx.enter_context(tc.tile_pool(name="x", bufs=NBUF))
            e_pool = ctx.enter_context(tc.tile_pool(name="exps", bufs=NBUF))
            res_pool = ctx.enter_context(tc.tile_pool(name="res", bufs=1))

            sid = const_pool.tile([P, N], dt)
            nc.sync.dma_start(sid[:], src_ids[:])
            vb = const_pool.tile([P, N], dt)
            nc.sync.dma_start(vb[:], vbias[:])
            tid = const_pool.tile([P, NTILES], dt)
            nc.sync.dma_start(tid[:], tgt_ids[:])
            res_n = res_pool.tile([P, NTILES], dt)
            res_d = res_pool.tile([P, NTILES], dt)

            # warm-up: absorb the const-DMA waits on each compute engine so
            # later instructions stay within their sync-wait budget
            warm = res_pool.tile([P, 6], dt)
            nc.vector.tensor_copy(warm[:, 0:1], sid[:, 0:1])
            nc.vector.tensor_copy(warm[:, 1:2], tid[:, 0:1])
            nc.vector.tensor_copy(warm[:, 2:3], vb[:, 0:1])
            nc.gpsimd.tensor_copy(warm[:, 3:4], vb[:, 0:1])

            # per-tile accumulators (a shared tile would add same-engine WAW
            # sem waits to the producing instructions)
            accn = [
                res_pool.tile([P, 1], dt, name=f"an{i}", tag=f"an{i}")
                for i in range(NTILES)
            ]
            accd = [
                res_pool.tile([P, 1], dt, name=f"ad{i}", tag=f"ad{i}")
                for i in range(NTILES)
            ]
            # per-tile scratch for the scalar-engine interposer copies
            def scratch(prefix):
                return [
                    res_pool.tile([P, 1], dt, name=f"{prefix}{i}", tag=f"{prefix}{i}")
                    for i in range(NTILES)
                ]

            sca, scc, scd, sce, scf, scg, sch = (
                scratch("sa"), scratch("scc"), scratch("sd"), scratch("se"),
                scratch("sf"), scratch("sg"), scratch("sh"),
            )

            load_insts = []
            ma_insts = []
            for i in range(NTILES):
                xt = x_pool.tile([P, N], dt)
                # pool-queue interposers: absorb the load's cross-engine waits
                # (exp's read of the recycled slot / the DMA lane's WAW) so
                # the SWDGE DMACopy keeps a single sync wait
                pre = []
                if i >= NBUF:
                    gA = nc.gpsimd.tensor_copy(scd[i][:], accd[i - NBUF][:])
                    pre.append(gA)
                if i >= 8:
                    gB = nc.gpsimd.tensor_copy(sce[i][:], vb[:, 0:1])
                    add_dep_helper(
                        gB.ins, load_insts[i - 8].ins, sync=True,
                        reason="absorb DMA lane WAW",
                    )
                    pre.append(gB)
                ld = getattr(nc, LOAD_ENGINE).dma_start(
                    xt[:], x[i * P:(i + 1) * P, :]
                )
                for g in pre:
                    add_dep_helper(
                        ld.ins, g.ins, sync=False,
                        reason="load ordered after wait-absorbing interposer",
                    )
                load_insts.append(ld)

                # mask out invalid src columns (in-place): x += vbias.
                # two same-engine absorber copies take the load-RAW and the
                # slot-WAW waits so the TensorTensor keeps <=1 sync wait
                eng = nc.gpsimd if i in MASK_POOL_TILES else nc.vector
                c1 = eng.tensor_copy(scf[i][:], vb[:, 0:1])
                add_dep_helper(
                    c1.ins, ld.ins, sync=True, reason="absorb load RAW"
                )
                mpre = [c1]
                if i >= NBUF:
                    c2 = eng.tensor_copy(scg[i][:], vb[:, 0:1])
                    add_dep_helper(
                        c2.ins, ma_insts[i - NBUF].ins, sync=True,
                        reason="absorb x-slot WAW",
                    )
                    mpre.append(c2)
                ma = eng.tensor_add(xt[:], xt[:], vb[:])
                for g in mpre:
                    add_dep_helper(
                        ma.ins, g.ins, sync=False,
                        reason="maskadd ordered after absorber",
                    )
                ma_insts.append(ma)

                # scalar-engine interposers: each absorbs one cross-engine
                # wait so the exp keeps a single sync wait (its self-WAW)
                exp_deps = [nc.scalar.copy(scc[i][:], xt[:, 0:1])]
                if i >= NBUF:
                    exp_deps.append(nc.scalar.copy(sca[i][:], accn[i - NBUF][:]))
                et = e_pool.tile([P, N], dt)
                exp = nc.scalar.activation(
                    et[:], xt[:], mybir.ActivationFunctionType.Exp,
                    scale=10.0, accum_out=accd[i][:],
                )
                for d in exp_deps:
                    add_dep_helper(
                        exp.ins, d.ins, sync=False,
                        reason="exp ordered after wait-absorbing interposer",
                    )

                # fused compare-multiply-accumulate; junk full-size output is
                # written in place over et (no extra junk tile => no WAW).
                # a DVE absorber copy takes the et-slot WAW (STT i-NBUF) wait
                spre = []
                if i >= NBUF:
                    vC = nc.vector.tensor_copy(sch[i][:], accn[i - NBUF][:])
                    spre.append(vC)
                stt = nc.vector.scalar_tensor_tensor(
                    out=et[:],
                    in0=sid[:],
                    scalar=tid[:, i:i + 1],
                    in1=et[:],
                    op0=mybir.AluOpType.is_equal,
                    op1=mybir.AluOpType.mult,
                    accum_out=accn[i][:],
                )
                for g in spre:
                    add_dep_helper(
                        stt.ins, g.ins, sync=False,
                        reason="STT ordered after et-slot WAW absorber",
                    )

            for i in range(NTILES):
                nc.vector.tensor_copy(res_n[:, i:i + 1], accn[i][:])
                nc.vector.tensor_copy(res_d[:, i:i + 1], accd[i][:])
            nc.sync.dma_start(numer[:], res_n[:])
            nc.sync.dma_start(denom[:], res_d[:])
    return nc


def _get_nc():
    key = (tuple(sorted(MASK_POOL_TILES)), NBUF, LOAD_ENGINE)
    if key not in _NC_CACHE:
        _NC_CACHE[key] = _build_nc()
    return _NC_CACHE[key]


def _color_ids(src, tgt):
    """Map each color row to a per-batch integer id via exact byte equality."""
    src_f = np.ascontiguousarray(src.reshape(B, -1, C))
    tgt_f = np.ascontiguousarray(tgt.reshape(B, -1, C))
    n_s = src_f.shape[1]
    src_ids = np.empty((B, n_s), np.float32)
    tgt_ids = np.empty((B, tgt_f.shape[1]), np.float32)
    for b in range(B):
        allc = np.ascontiguousarray(np.concatenate([src_f[b], tgt_f[b]], axis=0))
        view = allc.view([("", allc.dtype)] * C).reshape(-1)
        _, inv = np.unique(view, return_inverse=True)
        ids = inv.astype(np.float32)
        s_ids, t_ids = ids[:n_s].copy(), ids[n_s:].copy()
        s_ids[np.all(src_f[b] == PAD, axis=-1)] = -1.0
        t_ids[np.all(tgt_f[b] == PAD, axis=-1)] = -2.0
        src_ids[b], tgt_ids[b] = s_ids, t_ids
    return src_ids, tgt_ids


def kernel(seg_sim_map, seg_colors_src, seg_colors_tgt):
    from concourse.bass_utils import run_bass_kernel_spmd

    seg_sim_map = np.asarray(seg_sim_map, dtype=np.float32)
    src_ids, tgt_ids = _color_ids(
        np.asarray(seg_colors_src, np.float32), np.asarray(seg_colors_tgt, np.float32)
    )

    in_maps = []
    for c in range(NCORES):
        b, h = c // 2, c % 2
        rows = slice(h * ROWS, (h + 1) * ROWS)
        vb = np.where(src_ids[b] == -1.0, np.float32(MASKBIAS), np.float32(0.0))
        in_maps.append({
            "x": np.ascontiguousarray(seg_sim_map[b, rows, :]),
            "src_ids": np.ascontiguousarray(np.broadcast_to(src_ids[b], (P, N))),
            "vbias": np.ascontiguousarray(
                np.broadcast_to(vb.astype(np.float32), (P, N))
            ),
            # [p, i] = id of row i*P + p
            "tgt_ids": np.ascontiguousarray(tgt_ids[b, rows].reshape(NTILES, P).T),
        })

    trace = os.environ.get("KERNEL_PROFILE", "") == "1"
    nc = _get_nc()
    out = run_bass_kernel_spmd(nc, in_maps, list(range(NCORES)), trace=trace)
    if trace and out.exec_time_ns is not None:
        print(f"HW exec time: {out.exec_time_ns} ns")
        print(f"HW exec mean: {out.mean_exec_time_ns} ns")

    numer = np.empty((B, N), np.float32)
    denom = np.empty((B, N), np.float32)
    for c in range(NCORES):
        b, h = c // 2, c % 2
        rows = slice(h * ROWS, (h + 1) * ROWS)
        numer[b, rows] = out.results[c]["numer"].T.reshape(ROWS)
        denom[b, rows] = out.results[c]["denom"].T.reshape(ROWS)

    # host finalize, mirroring the reference ops in f32 (touches 16K scalars)
    p_gt = numer / denom
    nll = -np.log(p_gt + np.float32(EPS))
    m = (numer > 0).astype(np.float32)
    nll3 = nll.reshape(B, S_TGT, L_TGT)
    m3 = m.reshape(B, S_TGT, L_TGT)
    nvalid = m3.sum(-1)
    seg_loss = np.where(
        nvalid > 0, (nll3 * m3).sum(-1) / np.maximum(nvalid, np.float32(1.0)), 0.0
    ).astype(np.float32)
    cnt = int((nvalid > 0).sum())
    total = np.float32(seg_loss.sum(dtype=np.float32) / np.float32(max(cnt, 1)))
    return np.asarray(total, np.float32), np.asarray(cnt, np.int32)


# revision 30
# speedup vs baseline: 1.4082x; 1.4082x over previous
"""Masked-softmax cross-entropy loss on 8 Trainium2 cores.

Math: for each target row t (16384 rows of length 4096):
  numer[t] = sum_j exp(x[t,j]/tau) over valid src cols j whose color == tgt color t
  denom[t] = sum_j exp(x[t,j]/tau) over valid src cols j
  p_gt = numer/denom, nll = -log(p_gt + eps), rows with numer==0 are masked out.
Segment/count aggregation (32 segments) happens on host - it touches 16K scalars.

Sharding: core c takes half a batch: batch c//2, row-half c%2 (2048 rows).
All rows on a core share one batch => one src color-id row.

Device pipeline per 256-row chunk (two 128-row tiles side by side):
  DMA (swdge):  load x chunk [128, 8192] f32 (contiguous 4MB)
  ScalarE:      et = exp(10*x) -> bf16, accum_out -> denom_all  (per tile)
  DVE (bf16 2x mode) per tile, fused compare-multiply-accumulate STTs:
      numer   = sum((src_id == tgt_id[t]) * et)
      invsum  = sum((src_id == -1)        * et)   (invalid-column mass)
Host: denom = denom_all - invsum.
Colors are mapped to small integer ids on host (exact byte equality), so a
bf16 equality compare on device reproduces the reference's exact color match.
src pad -> id -1, tgt pad -> id -2 (never matches anything valid).

Sync-wait budget: this walrus allows very few sem waits per instruction
(1 for STT/DMA/CTRL). Tiny same-engine "interposer" copies absorb
cross-engine waits, and the kernel-tail drain is split into one drain per
proc. Absorbers sit on cheap queues (scalar/vector copies ~80-300ns; pool
only absorbs for the loads it issues).
"""

import os
import numpy as np

B = 4
S_TGT = 8
L_TGT = 512
C = 4
N = 4096          # src columns (= 8*512), also total tgt rows per batch
P = 128
ROWS = 2048       # tgt rows per core (half a batch)
NTILES = ROWS // P    # 16 result tiles
TPC = 2               # tiles per DMA chunk
NCHUNK = NTILES // TPC
NBUF = 3              # chunk buffer depth (slot reuse distance)
NCORES = 8
PAD = -1.0
EPS = 1e-15

_NC_CACHE = {}


def _patch_split_drain():
    """Split the kernel-tail drain's sem waits across several drain
    instructions (walrus rejects >1 sync wait on one CTRL instruction)."""
    import concourse.tile as tile
    from concourse.vector_clock import ScopedClock, VectorClock

    if getattr(tile.TileContext, "_split_drain_patched", False):
        return

    def _drain_and_barrier(self, tick_clock, wait_clock):
        g = tick_clock.global_clock
        n = len(g)
        for base in range(n):
            vec = [g[i] if i == base else 0 for i in range(n)]
            if not any(vec):
                continue
            d = self.nc.sync.drain()
            wait_clock.add_sem_waits(d.ins, ScopedClock({None: VectorClock(vec)}))
        self.nc.all_engine_barrier()
        popped = self.nc._tile_sem_poison_stack.pop()
        assert popped is self._sem_poison
        self.nc.clear_and_free_semaphores(list(self.sems.allocated().values()))
        self.nc.all_engine_barrier()

    tile.TileContext._drain_and_barrier = _drain_and_barrier
    tile.TileContext._split_drain_patched = True


def _build_nc():
    import concourse.bass as bass
    import concourse.mybir as mybir
    import concourse.tile as tile
    from concourse.tile_rust import add_dep_helper
    from contextlib import ExitStack

    _patch_split_drain()
    nc = bass.Bass()
    f32 = mybir.dt.float32
    bf16 = mybir.dt.bfloat16
    NW = N * TPC  # chunk width in f32 elements
    x = nc.declare_dram_parameter("x", [ROWS, N], f32, isOutput=False)
    src_ids = nc.declare_dram_parameter("src_ids", [P, N], bf16, isOutput=False)
    tgt_ids = nc.declare_dram_parameter("tgt_ids", [P, NTILES], bf16,
                                        isOutput=False)
    numer = nc.declare_dram_parameter("numer", [P, NTILES], f32, isOutput=True)
    denall = nc.declare_dram_parameter("denall", [P, NTILES], f32, isOutput=True)
    invsum = nc.declare_dram_parameter("invsum", [P, NTILES], f32, isOutput=True)

    with tile.TileContext(nc) as tc:
        with ExitStack() as ctx:
            const_pool = ctx.enter_context(tc.tile_pool(name="const", bufs=1))
            x_pool = ctx.enter_context(tc.tile_pool(name="x", bufs=NBUF))
            e_pool = ctx.enter_context(tc.tile_pool(name="exps", bufs=NBUF))
            res_pool = ctx.enter_context(tc.tile_pool(name="res", bufs=1))

            sid = const_pool.tile([P, N], bf16)
            nc.sync.dma_start(sid[:], src_ids[:])
            tid = const_pool.tile([P, NTILES], bf16)
            nc.sync.dma_start(tid[:], tgt_ids[:])
            res_n = res_pool.tile([P, NTILES], f32)
            res_d = res_pool.tile([P, NTILES], f32)
            res_i = res_pool.tile([P, NTILES], f32)

            # warm-up copies absorb the const-DMA waits per engine
            warm = res_pool.tile([P, 4], bf16)
            nc.vector.tensor_copy(warm[:, 0:1], sid[:, 0:1])
            nc.vector.tensor_copy(warm[:, 1:2], tid[:, 0:1])
            nc.scalar.copy(warm[:, 2:3], sid[:, 0:1])
            nc.gpsimd.tensor_copy(warm[:, 3:4], tid[:, 0:1])

            def scratch(prefix, dt_=f32):
                return [
                    res_pool.tile([P, 1], dt_, name=f"{prefix}{i}",
                                  tag=f"{prefix}{i}")
                    for i in range(NTILES)
                ]

            accn = scratch("an")
            accd = scratch("ad")
            acci = scratch("ai")
            sca, scc, scd, sce, scf, sch, sci = (
                scratch("sa"), scratch("scc"), scratch("sd"), scratch("se"),
                scratch("sf"), scratch("sh"), scratch("si"),
            )

            load_insts = []
            for ci in range(NCHUNK):
                xt = x_pool.tile([P, NW], f32)
                # pool-queue interposers: absorb the load's cross-engine
                # waits (scalar's reads of the recycled slot / the DMA lane
                # WAW) so the SWDGE DMACopy keeps a single sync wait
                pre = []
                if ci >= NBUF:
                    gA = nc.gpsimd.tensor_copy(
                        scd[ci][:], accd[(ci - NBUF) * TPC + TPC - 1][:]
                    )
                    pre.append(gA)
                    for k, old in enumerate(load_insts[ci - NBUF]):
                        gB = nc.gpsimd.tensor_copy(
                            (sce[ci] if k == 0 else scf[ci])[:], tid[:, 0:1]
                        )
                        add_dep_helper(
                            gB.ins, old.ins, sync=True,
                            reason="absorb DMA lane WAW",
                        )
                        pre.append(gB)
                lds = []
                base = ci * P * TPC
                for k in range(TPC):
                    ld = nc.gpsimd.dma_start(
                        xt[:, k * N:(k + 1) * N],
                        x[base + k * P:base + (k + 1) * P, :],
                    )
                    for g in pre:
                        add_dep_helper(
                            ld.ins, g.ins, sync=False,
                            reason="load ordered after wait absorber",
                        )
                    lds.append(ld)
                load_insts.append(lds)

                et = e_pool.tile([P, NW], bf16)
                for h in range(TPC):
                    i = ci * TPC + h
                    xs = xt[:, h * N:(h + 1) * N]
                    es = et[:, h * N:(h + 1) * N]

                    # scalar-side absorbers: DMA-lane wait + et-slot WAW
                    exp_deps = []
                    if h == 0:
                        exp_deps.append(nc.scalar.copy(scc[i][:], xt[:, 0:1]))
                    if ci >= NBUF:
                        exp_deps.append(
                            nc.scalar.copy(sca[i][:], accn[i - NBUF * TPC][:])
                        )
                    exp = nc.scalar.activation(
                        es, xs, mybir.ActivationFunctionType.Exp,
                        scale=10.0, accum_out=accd[i][:],
                    )
                    for d in exp_deps:
                        add_dep_helper(
                            exp.ins, d.ins, sync=False,
                            reason="exp ordered after wait absorber",
                        )

                    # DVE absorber for the et-slot WAW, then the two fused
                    # compare-multiply-accumulate STTs (junk out in-place)
                    spre = []
                    if ci >= NBUF:
                        vC = nc.vector.tensor_copy(
                            sch[i][:], accn[i - NBUF * TPC][:]
                        )
                        spre.append(vC)
                    stt1 = nc.vector.scalar_tensor_tensor(
                        out=es, in0=sid[:], scalar=tid[:, i:i + 1], in1=es,
                        op0=mybir.AluOpType.is_equal,
                        op1=mybir.AluOpType.mult,
                        accum_out=accn[i][:],
                    )
                    vD = nc.vector.tensor_copy(sci[i][:], accn[i][:])
                    stt2 = nc.vector.scalar_tensor_tensor(
                        out=es, in0=sid[:], scalar=-1.0, in1=es,
                        op0=mybir.AluOpType.is_equal,
                        op1=mybir.AluOpType.mult,
                        accum_out=acci[i][:],
                    )
                    for g in spre:
                        add_dep_helper(
                            stt1.ins, g.ins, sync=False,
                            reason="STT1 ordered after et-slot WAW absorber",
                        )
                    add_dep_helper(
                        stt2.ins, vD.ins, sync=False,
                        reason="STT2 ordered after STT1-WAW absorber",
                    )

            for i in range(NTILES):
                nc.vector.tensor_copy(res_n[:, i:i + 1], accn[i][:])
                nc.vector.tensor_copy(res_d[:, i:i + 1], accd[i][:])
                nc.vector.tensor_copy(res_i[:, i:i + 1], acci[i][:])
            nc.sync.dma_start(numer[:], res_n[:])
            nc.sync.dma_start(denall[:], res_d[:])
            nc.sync.dma_start(invsum[:], res_i[:])
    return nc


def _get_nc():
    key = (NBUF, TPC)
    if key not in _NC_CACHE:
        _NC_CACHE[key] = _build_nc()
    return _NC_CACHE[key]


def _color_ids(src, tgt):
    """Map each color row to a per-batch integer id via exact byte equality."""
    src_f = np.ascontiguousarray(src.reshape(B, -1, C))
    tgt_f = np.ascontiguousarray(tgt.reshape(B, -1, C))
    n_s = src_f.shape[1]
    src_ids = np.empty((B, n_s), np.float32)
    tgt_ids = np.empty((B, tgt_f.shape[1]), np.float32)
    for b in range(B):
        allc = np.ascontiguousarray(np.concatenate([src_f[b], tgt_f[b]], axis=0))
        view = allc.view([("", allc.dtype)] * C).reshape(-1)
        _, inv = np.unique(view, return_inverse=True)
        ids = inv.astype(np.float32)
        s_ids, t_ids = ids[:n_s].copy(), ids[n_s:].copy()
        s_ids[np.all(src_f[b] == PAD, axis=-1)] = -1.0
        t_ids[np.all(tgt_f[b] == PAD, axis=-1)] = -2.0
        src_ids[b], tgt_ids[b] = s_ids, t_ids
    return src_ids, tgt_ids


def kernel(seg_sim_map, seg_colors_src, seg_colors_tgt):
    import ml_dtypes
    from concourse.bass_utils import run_bass_kernel_spmd

    bf16 = ml_dtypes.bfloat16
    seg_sim_map = np.asarray(seg_sim_map, dtype=np.float32)
    src_ids, tgt_ids = _color_ids(
        np.asarray(seg_colors_src, np.float32), np.asarray(seg_colors_tgt, np.float32)
    )

    in_maps = []
    for c in range(NCORES):
        b, h = c // 2, c % 2
        rows = slice(h * ROWS, (h + 1) * ROWS)
        in_maps.append({
            "x": np.ascontiguousarray(seg_sim_map[b, rows, :]),
            "src_ids": np.ascontiguousarray(
                np.broadcast_to(src_ids[b].astype(bf16), (P, N))
            ),
            # [p, i] = id of row i*P + p
            "tgt_ids": np.ascontiguousarray(
                tgt_ids[b, rows].reshape(NTILES, P).T.astype(bf16)
            ),
        })

    trace = os.environ.get("KERNEL_PROFILE", "") == "1"
    nc = _get_nc()
    out = run_bass_kernel_spmd(nc, in_maps, list(range(NCORES)), trace=trace)
    if trace and out.exec_time_ns is not None:
        print(f"HW exec time: {out.exec_time_ns} ns")
        print(f"HW exec mean: {out.mean_exec_time_ns} ns")

    numer = np.empty((B, N), np.float32)
    denom = np.empty((B, N), np.float32)
    for c in range(NCORES):
        b, h = c // 2, c % 2
        rows = slice(h * ROWS, (h + 1) * ROWS)
        r = out.results[c]
        numer[b, rows] = r["numer"].T.reshape(ROWS)
        denom[b, rows] = (r["denall"] - r["invsum"]).T.reshape(ROWS)

    # host finalize, mirroring the reference ops in f32 (touches 16K scalars)
    p_gt = numer / denom
    nll = -np.log(p_gt + np.float32(EPS))
    m = (numer > 0).astype(np.float32)
    nll3 = nll.reshape(B, S_TGT, L_TGT)
    m3 = m.reshape(B, S_TGT, L_TGT)
    nvalid = m3.sum(-1)
    seg_loss = np.where(
        nvalid > 0, (nll3 * m3).sum(-1) / np.maximum(nvalid, np.float32(1.0)), 0.0
    ).astype(np.float32)
    cnt = int((nvalid > 0).sum())
    total = np.float32(seg_loss.sum(dtype=np.float32) / np.float32(max(cnt, 1)))
    return np.asarray(total, np.float32), np.asarray(cnt, np.int32)
